# revision 9
# baseline (speedup 1.0000x reference)
"""L2 self-attention (q==k) Bass/Tile kernel for 8 TRN2 NeuronCores.

Sharding: core c = 2*b + g handles batch b and head-group g (8 of 16 heads).
Each core computes the partial output  attn_out_bg @ Wo[g*512:(g+1)*512, :].
Host sums the two partials per batch and adds bo.

Math per head (s = DIM_HEAD**-0.5):
  sim_ij = -s*||q_i - q_j||^2 = 2s*AB_ij - s*AA_i - s*AA_j
  softmax rows are invariant to the per-row constant  -s*AA_i, so
  P_ij = exp(2s*AB_ij - s*AA_j) / sum_j exp(2s*AB_ij - s*AA_j)
  Exponent is <= 0 + bounded (2ab <= a^2+b^2), no max-subtraction needed.
  out = P @ v ; den folded in as a 65th (ones) column of v.
"""

import numpy as np

B, N, D = 4, 2048, 1024
HEADS, DIM_HEAD = 16, 64
INNER = HEADS * DIM_HEAD
SCALE = DIM_HEAD ** -0.5

NCORES = 8
NH = 8            # heads per core
DL = NH * DIM_HEAD  # 512 local inner dims
KT = D // 128     # 8 full contraction tiles for projections
JT = N // 128     # 16 key tiles
IC = 2            # i-chunks of 1024 columns
ICW = N // IC     # 1024

_CACHE = {}


def _build_nc():
    import concourse.bacc as bacc
    import concourse.bass as bass
    import concourse.mybir as mybir
    import concourse.tile as tile
    from concourse.masks import make_identity

    f32 = mybir.dt.float32
    X = mybir.AxisListType.X
    EXP = mybir.ActivationFunctionType.Exp

    nc = bacc.Bacc("TRN2", target_bir_lowering=False, debug=False,
                   num_devices=NCORES)

    xTa_d = nc.dram_tensor("xTa", [D + 1, N], f32, kind="ExternalInput")
    wqa_d = nc.dram_tensor("wqa", [D + 1, DL], f32, kind="ExternalInput")
    wva_d = nc.dram_tensor("wva", [D + 1, DL], f32, kind="ExternalInput")
    wo_d = nc.dram_tensor("wo", [DL, D], f32, kind="ExternalInput")
    part_d = nc.dram_tensor("part", [N, D], f32, kind="ExternalOutput")
    xTa = xTa_d.ap()
    wqa = wqa_d.ap()
    wva = wva_d.ap()
    wo_ap = wo_d.ap()
    part = part_d.ap()

    with tile.TileContext(nc) as tc, \
         tc.tile_pool(name="persist", bufs=1) as persist:
        # ---- persistent tensors (whole-kernel lifetime) ----
        qT = [persist.tile([128, N], f32, tag=f"qT{t}", name=f"qT{t}")
              for t in range(4)]
        v_aug = persist.tile([128, JT, NH * 65], f32, tag="v_aug", name="v_aug")
        aa = persist.tile([128, NH * JT], f32, tag="aa", name="aa")
        ident = persist.tile([128, 128], f32, tag="ident", name="ident")

        make_identity(nc, ident)
        nc.vector.tensor_scalar_mul(ident, ident, -SCALE)
        nc.gpsimd.memset(v_aug, 0.0)
        for h in range(NH):
            nc.vector.memset(v_aug[:, :, h * 65 + 64 : h * 65 + 65], 1.0)

        # ---- phase 1: projections ----
        with tc.tile_pool(name="pin", bufs=1) as pin:
            xt = [pin.tile([128, N], f32, tag=f"xt{k}", name=f"xt{k}") for k in range(KT)]
            xt.append(pin.tile([1, N], f32, tag="xt_ones", name="xt_ones"))
            wq = [pin.tile([128, DL], f32, tag=f"wq{k}", name=f"wq{k}") for k in range(KT)]
            wq.append(pin.tile([1, DL], f32, tag="wq_b", name="wq_b"))
            wv = [pin.tile([128, DL], f32, tag=f"wv{k}", name=f"wv{k}") for k in range(KT)]
            wv.append(pin.tile([1, DL], f32, tag="wv_b", name="wv_b"))
            for k in range(KT):
                sl = slice(k * 128, (k + 1) * 128)
                nc.sync.dma_start(out=xt[k], in_=xTa[sl, :])
                nc.sync.dma_start(out=wq[k], in_=wqa[sl, :])
                nc.sync.dma_start(out=wv[k], in_=wva[sl, :])
            nc.sync.dma_start(out=xt[KT], in_=xTa[D : D + 1, :])
            nc.sync.dma_start(out=wq[KT], in_=wqa[D : D + 1, :])
            nc.sync.dma_start(out=wv[KT], in_=wva[D : D + 1, :])

            # qT[d, i] : lhsT = wqa[:, d-tile], rhs = xTa[:, i-chunk]
            with tc.tile_pool(name="qps", bufs=2, space="PSUM") as qps:
                for dt in range(4):
                    ps = qps.tile([128, N], f32, tag="qproj")
                    dsl = slice(dt * 128, (dt + 1) * 128)
                    for k in range(KT + 1):
                        for nck in range(4):
                            nsl = slice(nck * 512, (nck + 1) * 512)
                            nc.tensor.matmul(ps[:, nsl], lhsT=wq[k][:, dsl],
                                             rhs=xt[k][:, nsl],
                                             start=(k == 0), stop=(k == KT))
                    nc.vector.tensor_copy(qT[dt], ps)

            # v[i, d] : lhsT = xTa[:, i-tile], rhs = wva ; scatter into v_aug
            with tc.tile_pool(name="vps", bufs=4, space="PSUM") as vps:
                for it in range(JT):
                    ps = vps.tile([128, DL], f32, tag="vproj")
                    isl = slice(it * 128, (it + 1) * 128)
                    for k in range(KT + 1):
                        nc.tensor.matmul(ps, lhsT=xt[k][:, isl], rhs=wv[k],
                                         start=(k == 0), stop=(k == KT))
                    src = ps.rearrange("p (h w) -> p h w", w=64)
                    dst = v_aug[:, it, :].rearrange("p (h w) -> p h w", w=65)
                    nc.vector.tensor_copy(dst[:, :, 0:64], src)

        # allocated after the projection pool closes so phase-1 SBUF peak
        # (xt/wq/wv tiles) and these never coexist in the address map
        p2 = tc.alloc_tile_pool(name="persist2", bufs=1)
        ot = [p2.tile([128, N], f32, tag=f"ot{t}", name=f"ot{t}")
              for t in range(4)]
        wo_sb = [p2.tile([128, D], f32, tag=f"wo{t}", name=f"wo{t}")
                 for t in range(4)]
        for t in range(4):
            nc.sync.dma_start(out=wo_sb[t], in_=wo_ap[t * 128 : (t + 1) * 128, :])

        # ---- phase 2a: AA diag pass:  aa[:, h*JT+jt] = -s * ||q_j||^2 ----
        with tc.tile_pool(name="dps", bufs=4, space="PSUM") as dps, \
             tc.tile_pool(name="dsb", bufs=4) as dsb:
            for h in range(NH):
                dt, half = divmod(h, 2)
                rows = slice(half * 64, half * 64 + 64)
                for jt in range(JT):
                    jsl = slice(jt * 128, (jt + 1) * 128)
                    ps = dps.tile([128, 128], f32, tag="diag")
                    nc.tensor.matmul(ps, lhsT=qT[dt][rows, jsl],
                                     rhs=qT[dt][rows, jsl],
                                     start=True, stop=True)
                    sc = dsb.tile([128, 128], f32, tag="dsc")
                    nc.vector.tensor_mul(sc, ps, ident)
                    col = h * JT + jt
                    nc.vector.reduce_sum(out=aa[:, col : col + 1], in_=sc,
                                         axis=X)

        # ---- phase 2b: attention per head ----
        with tc.tile_pool(name="sps", bufs=2, space="PSUM") as sps, \
             tc.tile_pool(name="nps", bufs=2, space="PSUM") as nps, \
             tc.tile_pool(name="gp", bufs=3) as gp, \
             tc.tile_pool(name="nrm", bufs=2) as nrm:
            for h in range(NH):
                dt, half = divmod(h, 2)
                rows = slice(half * 64, half * 64 + 64)
                vsl = slice(h * 65, (h + 1) * 65)
                for ic in range(IC):
                    i0 = ic * ICW
                    nm = nps.tile([65, ICW], f32, tag="num")
                    gs = [None] * JT
                    for jt in range(JT):
                        jsl = slice(jt * 128, (jt + 1) * 128)
                        sp = sps.tile([128, ICW], f32, tag="scores")
                        for q in range(2):
                            qsl = slice(q * 512, (q + 1) * 512)
                            nc.tensor.matmul(
                                sp[:, qsl], lhsT=qT[dt][rows, jsl],
                                rhs=qT[dt][rows, i0 + q * 512 : i0 + (q + 1) * 512],
                                start=True, stop=True)
                        g = gp.tile([128, ICW], f32, tag="gtile")
                        col = h * JT + jt
                        nc.scalar.activation(out=g, in_=sp, func=EXP,
                                             bias=aa[:, col : col + 1],
                                             scale=2.0 * SCALE)
                        gs[jt] = g
                        # one-step software skew: num(jt-1) after S(jt)/exp(jt)
                        if jt > 0:
                            for q in range(2):
                                qsl = slice(q * 512, (q + 1) * 512)
                                nc.tensor.matmul(nm[:, qsl], lhsT=v_aug[:, jt - 1, vsl],
                                                 rhs=gs[jt - 1][:, qsl],
                                                 start=(jt == 1), stop=False)
                    for q in range(2):
                        qsl = slice(q * 512, (q + 1) * 512)
                        nc.tensor.matmul(nm[:, qsl], lhsT=v_aug[:, JT - 1, vsl],
                                         rhs=gs[JT - 1][:, qsl],
                                         start=False, stop=True)
                    # normalize: ot[rows, i0:i0+ICW] = nm[0:64] / nm[64]
                    rd = nrm.tile([1, ICW], f32, tag="rden", name="rden")
                    nc.vector.reciprocal(rd, nm[64:65, :])
                    rdb = nrm.tile([64, ICW], f32, tag="rdenb", name="rdenb")
                    nc.gpsimd.partition_broadcast(rdb, rd)
                    nc.vector.tensor_mul(ot[dt][rows, i0 : i0 + ICW],
                                         nm[0:64, :], rdb)

        # ---- phase 3: output projection ----
        with tc.tile_pool(name="ops", bufs=2, space="PSUM") as ops, \
             tc.tile_pool(name="osb", bufs=3) as osb:
            for it in range(JT):
                isl = slice(it * 128, (it + 1) * 128)
                ps = ops.tile([128, 1024], f32, tag="oproj")
                for ock in range(2):
                    osl = slice(ock * 512, (ock + 1) * 512)
                    for dlt in range(4):
                        nc.tensor.matmul(ps[:, osl], lhsT=ot[dlt][:, isl],
                                         rhs=wo_sb[dlt][:, osl],
                                         start=(dlt == 0), stop=(dlt == 3))
                ob = osb.tile([128, 1024], f32, tag="obuf", name="obuf")
                nc.vector.tensor_copy(ob, ps)
                nc.sync.dma_start(out=part[isl, :], in_=ob)

        p2.release()

    nc.compile()
    return nc


def _get_nc():
    if "nc" not in _CACHE:
        _CACHE["nc"] = _build_nc()
    return _CACHE["nc"]


def make_in_maps(x, Wq, bq, Wv, bv, Wo, bo):
    x = np.asarray(x, dtype=np.float32)
    Wq = np.asarray(Wq, dtype=np.float32)
    bq = np.asarray(bq, dtype=np.float32)
    Wv = np.asarray(Wv, dtype=np.float32)
    bv = np.asarray(bv, dtype=np.float32)
    Wo = np.asarray(Wo, dtype=np.float32)
    in_maps = []
    for c in range(NCORES):
        b, g = divmod(c, 2)
        gsl = slice(g * DL, (g + 1) * DL)
        xTa = np.concatenate([np.ascontiguousarray(x[b].T),
                              np.ones((1, N), np.float32)], axis=0)
        wqa = np.concatenate([Wq[:, gsl], bq[gsl][None, :]], axis=0)
        wva = np.concatenate([Wv[:, gsl], bv[gsl][None, :]], axis=0)
        in_maps.append({
            "xTa": np.ascontiguousarray(xTa),
            "wqa": np.ascontiguousarray(wqa),
            "wva": np.ascontiguousarray(wva),
            "wo": np.ascontiguousarray(Wo[gsl, :]),
        })
    return in_maps


def combine_parts(parts, bo):
    bo = np.asarray(bo, dtype=np.float32)
    out = np.empty((B, N, D), np.float32)
    for b in range(B):
        out[b] = parts[2 * b] + parts[2 * b + 1] + bo
    return out


def kernel(x, Wq, bq, Wv, bv, Wo, bo):
    from concourse.bass_utils import run_bass_kernel_spmd

    nc = _get_nc()
    in_maps = make_in_maps(x, Wq, bq, Wv, bv, Wo, bo)
    res = run_bass_kernel_spmd(nc, in_maps, core_ids=list(range(NCORES)))
    parts = [r["part"] for r in res.results]
    return combine_parts(parts, bo)
